# revision 1
# baseline (speedup 1.0000x reference)
"""Sparse (block-local) attention for B=2, Sq=2048, Sk=4096, D=1024, H=16.

Each query i attends to exactly keys {2i, 2i+1} (Sk/Sq == 2, no remainder),
so softmax is over 2 scores -> p1 = sigmoid((s1-s2)*scale), p2 = sigmoid((s2-s1)*scale).

Distribution: sequence-parallel over (batch, query-block). 8 cores, each takes
512 contiguous queries of one batch plus the matching 1024 contiguous keys.
No collectives needed; outputs are concatenated on the host.

Per-core device kernel (all matmuls bf16 with fp32 PSUM accumulation):
  Q  = x_s  @ Wq^T           row-major   [512, 1024]
  K  = c_perm @ Wk^T         row-major   [1024, 1024] (keys permuted even|odd)
  V  = c_perm @ Wv^T         row-major   [1024, 1024]
  s1/s2 row-wise dots on DVE (mul + grouped reduce per 64-dim head)
  p1/p2 on ACT (sigmoid), AV combine on DVE -> att [512, 1024]
  att^T via PE transposes, O = att @ Wo^T, DMA out.

Host side only reshapes/shards/casts: feature-major + partition-major tiled
layouts, keys permuted even|odd, cast to bf16, concatenate core outputs.

Engine budget: PE ~89us (the bottleneck), ACT does all projection-PSUM
copies so DVE is free to run attention as soon as its inputs land.
"""

import sys

for _p in ("/opt/trn_rl_repo",):
    if _p not in sys.path:
        sys.path.append(_p)

import numpy as np
import ml_dtypes

import concourse.bass as bass
import concourse.mybir as mybir
import concourse.tile as tile
from concourse import bacc
from concourse.bass_utils import run_bass_kernel_spmd
from concourse.masks import make_identity
from concourse.tile_rust import add_dep_helper

B, SQ, SK, D, H, HD = 2, 2048, 4096, 1024, 16, 64
N_CORES = 8
QL = B * SQ // N_CORES       # 512 queries per core
KL = 2 * QL                  # 1024 keys per core
QT = QL // 128               # 4 query tiles
NB = 512                     # psum bank width (fp32)
JT = D // NB                 # 2 output-column blocks per projection
DT = D // 128                # 8 feature tiles
SCALE = 1.0 / float(np.sqrt(HD))

FB = mybir.dt.bfloat16
F32 = mybir.dt.float32
F8 = mybir.dt.float8e4
BF = ml_dtypes.bfloat16
E4M3 = ml_dtypes.float8_e4m3fn
WSCALE = 32.0


def _build(kd_tiles: int, with_bo: bool, fp8: bool = False):
    """Build + finalize the per-core Bacc graph (SPMD: same graph on 8 cores).

    fp8=True (bias-free path only): Wq ships as fp8 e4m3 pre-scaled by
    WSCALE (folded back in the sigmoid scale), and the front DMA chunks are
    fine-grained whole tensors (fat descriptor rows) split across both
    hwdge rings with NO completion chains — phase order is pinned via
    tile_wait_until instead. Cuts the first-matmul-group data from 1.84MB
    (one fragmented chunk) to ~0.76MB across two rings.
    """
    if fp8:
        return _build_fp8()
    nc = bacc.Bacc("TRN2", target_bir_lowering=False)

    # All activation/weight inputs are host-arranged partition-major:
    # tensor[p, t, n] = logical[t*128 + p, n], so DMA descriptors are
    # per-partition contiguous. Inputs are merged by NEED ORDER and the
    # DMA chain is gated so each phase gets full HBM bandwidth:
    # Key algebraic cuts: with exactly 2 keys per query, softmax only needs the
    # score DIFFERENCE, and k_even - k_odd = (c_even - c_odd) @ Wk^T is linear,
    # so the K projection runs on c_diff = c_even - c_odd (512 rows, not 1024).
    # Likewise att = v_odd + p1 * (v_even - v_odd) reuses c_diff for V, and the
    # v_odd term folds through the output projection with a host-precomputed
    # weight product Wvo = Wo @ Wv:
    #   out = c_odd @ Wvo^T + (p1 * Vd) @ Wo^T,  Vd = c_diff @ Wv^T
    # so V_odd is never materialized.
    # DMA need-order:
    #   xw0 = xT[:, qt0-2] | wq[:, 0:512] -> Q's first jb0 groups
    #   xw1 = xT[:, qt3] | wq[:, 512:1024] -> rest of Q
    #   ck  = c_diffT | wk        -> Kd projection
    #   cv  = c_oddT | wv         -> Vd projection + O's pure half
    #   woo = wo | wvo            -> output projection
    X0Q = 3 * 128               # x columns (queries) in xw0
    xw0 = nc.dram_tensor("xw0", [128, kd_tiles, X0Q + NB], FB,
                         kind="ExternalInput")
    xw1 = nc.dram_tensor("xw1", [128, kd_tiles, (QL - X0Q) + (D - NB)], FB,
                         kind="ExternalInput")
    ck = nc.dram_tensor("ck", [128, kd_tiles, QL + D], FB, kind="ExternalInput")
    cv = nc.dram_tensor("cv", [128, kd_tiles, QL + D], FB, kind="ExternalInput")
    woo = nc.dram_tensor("woo", [128, kd_tiles, 2 * D], FB,
                         kind="ExternalInput")
    bo = None
    if with_bo:
        bo = nc.dram_tensor("bo", [1, D], F32, kind="ExternalInput")
    out = nc.dram_tensor("out", [128, QT, D], F32, kind="ExternalOutput")

    with tile.TileContext(nc) as tc:
        with (
            tc.tile_pool(name="ins", bufs=1) as ins,
            tc.tile_pool(name="acts", bufs=1) as acts,
            tc.tile_pool(name="att", bufs=4) as att,
            tc.tile_pool(name="outs", bufs=4) as outs,
            tc.tile_pool(name="psum", bufs=6, space="PSUM") as psum,
            tc.tile_pool(name="psum_tr", bufs=2, space="PSUM") as psum_tr,
        ):
            # ---- inputs to SBUF (need-order chained DMAs) ------------------
            xw0_sb = ins.tile([128, kd_tiles, X0Q + NB], FB)
            xw1_sb = ins.tile([128, kd_tiles, (QL - X0Q) + (D - NB)], FB)
            ck_sb = ins.tile([128, kd_tiles, QL + D], FB)
            cv_sb = ins.tile([128, kd_tiles, QL + D], FB)
            woo_sb = ins.tile([128, kd_tiles, 2 * D], FB)
            ident = ins.tile([128, 128], FB)

            # xw0 split across both physical HWDGE rings (sync + scalar) to
            # halve the descriptor fan-out latency of the first transfer
            h0 = (X0Q + NB) // 2
            d0a = nc.sync.dma_start(out=xw0_sb[:, :, 0:h0], in_=xw0[:, :, 0:h0])
            d0b = nc.scalar.dma_start(out=xw0_sb[:, :, h0:], in_=xw0[:, :, h0:])
            d1 = nc.sync.dma_start(out=xw1_sb, in_=xw1[:])
            d2 = nc.sync.dma_start(out=ck_sb, in_=ck[:])
            d3 = nc.sync.dma_start(out=cv_sb, in_=cv[:])
            d4 = nc.sync.dma_start(out=woo_sb, in_=woo[:])
            # xw0 alone gets full bandwidth; then xw1 (small) and ck share;
            # cv after both; wo last
            for d0x in (d0a, d0b):
                add_dep_helper(d1.ins, d0x.ins, sync=True)
                add_dep_helper(d2.ins, d0x.ins, sync=True)
            add_dep_helper(d3.ins, d1.ins, sync=True)
            add_dep_helper(d3.ins, d2.ins, sync=True)
            add_dep_helper(d4.ins, d3.ins, sync=True)
            bo_sb = None
            if with_bo:
                bo_sb = ins.tile([128, D], F32)
                d5 = nc.sync.dma_start(out=bo_sb,
                                       in_=bo[:].to_broadcast((128, D)))
                add_dep_helper(d5.ins, d3.ins, sync=True)
            make_identity(nc, ident)

            # PE warm-up: dummy matmuls during the DMA head keep HAM busy so
            # the real stream starts at full clock, at zero wall-clock cost.
            warm = ins.tile([128, 128], FB)
            nc.vector.memset(warm, 1.0)
            wps = psum_tr.tile([128, 128], F32, tag="tr")
            for _ in range(110):
                nc.tensor.matmul(wps, lhsT=warm, rhs=warm, start=True, stop=True)

            def x_slice(kd, col0):
                if col0 < X0Q:
                    return xw0_sb[:, kd, col0:col0 + 128]
                c = col0 - X0Q
                return xw1_sb[:, kd, c:c + 128]

            def wq_slice(kd, jb):
                if jb == 0:
                    return xw0_sb[:, kd, X0Q:X0Q + NB]
                c = (QL - X0Q) + (jb - 1) * NB
                return xw1_sb[:, kd, c:c + NB]

            def cdiff_slice(kd, col0):
                return ck_sb[:, kd, col0:col0 + 128]

            def wk_slice(kd, jb):
                return ck_sb[:, kd, QL + jb * NB:QL + (jb + 1) * NB]

            def codd_slice(kd, col0):
                return cv_sb[:, kd, col0:col0 + 128]

            def wv_slice(kd, jb):
                return cv_sb[:, kd, QL + jb * NB:QL + (jb + 1) * NB]

            # ---- projections (psum copies all on ACT) ----------------------
            q_sb = acts.tile([128, QT, D], FB)           # Q row-major
            kd_sb = acts.tile([128, QT, D], FB)          # Kd = c_diff @ Wk^T
            v_sb = acts.tile([128, QT, D], FB)           # Vd = c_diff @ Wv^T

            def mm_one(dst_tile, dst_idx, jb, lhs_fn, rhs_fn, nkd=kd_tiles):
                ps = psum.tile([128, NB], F32, tag="mm")
                for kd in range(nkd):
                    nc.tensor.matmul(
                        ps,
                        lhsT=lhs_fn(kd),
                        rhs=rhs_fn(kd, jb),
                        start=(kd == 0),
                        stop=(kd == nkd - 1),
                    )
                nc.scalar.copy(dst_tile[:, dst_idx, jb * NB:(jb + 1) * NB], ps)

            def mm_group(dst_tile, dst_idx, lhs_fn, rhs_fn):
                for jb in range(JT):
                    mm_one(dst_tile, dst_idx, jb, lhs_fn, rhs_fn)

            # attention state per query tile: av = p1 * Vd (the v_odd term is
            # folded into the output projection via Wvo)
            av_sb = acts.tile([128, QT, D], FB)

            def attention(qt):
                # ds = rowdot(q, kd) per head; p1 = sigmoid(scale*ds);
                # av = p1 * v_diff
                qv = q_sb[:, qt, :]
                kdv = kd_sb[:, qt, :]
                pe = att.tile([128, H, HD], FB, tag="prod")
                nc.vector.tensor_mul(pe.rearrange("p h e -> p (h e)"), qv, kdv)
                ds = att.tile([128, H], F32, tag="s")
                nc.vector.reduce_sum(out=ds, in_=pe, axis=mybir.AxisListType.X)
                p1 = att.tile([128, H], F32, tag="s")
                nc.scalar.activation(p1, ds, mybir.ActivationFunctionType.Sigmoid,
                                     scale=SCALE)
                vd = v_sb[:, qt, :].rearrange("p (h e) -> p h e", h=H)
                nc.vector.tensor_mul(
                    av_sb[:, qt, :].rearrange("p (h e) -> p h e", h=H),
                    vd, p1.to_broadcast((128, H, HD)))

            # Q first, jb-outer: the jb0 groups only need xw0 (the first DMA),
            # jb1 groups unblock when xw1 lands
            for jb in range(JT):
                for qt in range(QT):
                    mm_one(q_sb, qt, jb,
                           lambda kd, qt=qt: x_slice(kd, qt * 128), wq_slice)
            # Kd for all qt (needs only ck), then Vd per qt (needs cv);
            # attention(qt) emitted one qt later so its ACT sigmoid never
            # stalls the projection-copy stream
            for qt in range(QT):
                mm_group(kd_sb, qt,
                         lambda kd, qt=qt: cdiff_slice(kd, qt * 128), wk_slice)
            for qt in range(QT):
                mm_group(v_sb, qt,
                         lambda kd, qt=qt: cdiff_slice(kd, qt * 128), wv_slice)
                if qt >= 1:
                    attention(qt - 1)
            attention(QT - 1)

            # ---- transpose att -> attT (copies on ACT), O groups interleaved
            avT_sb = acts.tile([128, DT, QL], FB)        # att^T feature-major

            def transposes(qt):
                for db in range(DT):
                    tp = psum_tr.tile([128, 128], FB, tag="tr")
                    nc.tensor.transpose(tp, av_sb[:, qt, db * 128:(db + 1) * 128],
                                        ident)
                    nc.scalar.copy(avT_sb[:, db, qt * 128:(qt + 1) * 128], tp)

            def o_group(qt):
                # out[qt] = c_odd @ Wvo^T (pure half, no attention dep)
                #         + av @ Wo^T     (attention half)
                # accumulated into one psum bank per jb; the pure half runs
                # while ACT is still copying this qt's avT tiles
                pss = [psum.tile([128, NB], F32, tag="mm", name=f"psg{jb}") for jb in range(JT)]
                for jb in range(JT):
                    for kd in range(kd_tiles):
                        nc.tensor.matmul(
                            pss[jb],
                            lhsT=codd_slice(kd, qt * 128),
                            rhs=woo_sb[:, kd, D + jb * NB:D + (jb + 1) * NB],
                            start=(kd == 0),
                            stop=False,
                        )
                    for kd in range(DT):
                        nc.tensor.matmul(
                            pss[jb],
                            lhsT=avT_sb[:, kd, qt * 128:(qt + 1) * 128],
                            rhs=woo_sb[:, kd, jb * NB:(jb + 1) * NB],
                            start=False,
                            stop=(kd == DT - 1),
                        )
                for jb in range(JT):
                    o_t = outs.tile([128, NB], F32, tag="o")
                    if with_bo:
                        nc.vector.tensor_add(o_t, pss[jb],
                                             bo_sb[:, jb * NB:(jb + 1) * NB])
                    elif jb % 2 == 0:
                        # jb0 on ACT, jb1 on DVE so the final group's two
                        # copies run in parallel right after the last matmul
                        nc.scalar.copy(o_t, pss[jb])
                    else:
                        nc.vector.tensor_copy(o_t, pss[jb])
                    nc.sync.dma_start(out=out[:, qt, jb * NB:(jb + 1) * NB],
                                      in_=o_t)

            # PE order: T0 T1 O0 T2 O1 T3 O2 O3 — keeps PE fed while ACT
            # copies each avT tile group
            transposes(0)
            transposes(1)
            o_group(0)
            transposes(2)
            o_group(1)
            transposes(3)
            o_group(2)
            o_group(3)

    nc.finalize()
    return nc


def _build_fp8():
    """Bias-free fast path: fp8 Wq, fine-grained unchained front DMA."""
    nc = bacc.Bacc("TRN2", target_bir_lowering=False)
    kd_tiles = DT

    wq0 = nc.dram_tensor("wq0", [128, kd_tiles, NB], F8, kind="ExternalInput")
    wq1 = nc.dram_tensor("wq1", [128, kd_tiles, NB], F8, kind="ExternalInput")
    xq0 = nc.dram_tensor("xq0", [128, kd_tiles, 128], FB,
                         kind="ExternalInput")
    xq123 = nc.dram_tensor("xq123", [128, kd_tiles, QL - 128], FB,
                           kind="ExternalInput")
    cdf = nc.dram_tensor("cdf", [128, kd_tiles, QL], FB, kind="ExternalInput")
    wk0 = nc.dram_tensor("wk0", [128, kd_tiles, NB], FB, kind="ExternalInput")
    wk1 = nc.dram_tensor("wk1", [128, kd_tiles, NB], FB, kind="ExternalInput")
    wv0 = nc.dram_tensor("wv0", [128, kd_tiles, NB], FB, kind="ExternalInput")
    wv1 = nc.dram_tensor("wv1", [128, kd_tiles, NB], FB, kind="ExternalInput")
    cod = nc.dram_tensor("cod", [128, kd_tiles, QL], FB, kind="ExternalInput")
    wvo = nc.dram_tensor("wvo", [128, kd_tiles, D], FB, kind="ExternalInput")
    wo = nc.dram_tensor("wo", [128, kd_tiles, D], FB, kind="ExternalInput")
    out = nc.dram_tensor("out", [128, QT, D], F32, kind="ExternalOutput")

    with tile.TileContext(nc) as tc:
        with (
            tc.tile_pool(name="ins", bufs=1) as ins,
            tc.tile_pool(name="acts", bufs=1) as acts,
            tc.tile_pool(name="att", bufs=4) as att,
            tc.tile_pool(name="outs", bufs=4) as outs,
            tc.tile_pool(name="psum", bufs=6, space="PSUM") as psum,
            tc.tile_pool(name="psum_tr", bufs=2, space="PSUM") as psum_tr,
        ):
            wq0_sb = ins.tile([128, kd_tiles, NB], F8)
            wq1_sb = ins.tile([128, kd_tiles, NB], F8)
            xq0_sb = ins.tile([128, kd_tiles, 128], FB)
            xq123_sb = ins.tile([128, kd_tiles, QL - 128], FB)
            cdf_sb = ins.tile([128, kd_tiles, QL], FB)
            wk0_sb = ins.tile([128, kd_tiles, NB], FB)
            wk1_sb = ins.tile([128, kd_tiles, NB], FB)
            wv0_sb = ins.tile([128, kd_tiles, NB], FB)
            wv1_sb = ins.tile([128, kd_tiles, NB], FB)
            cod_sb = ins.tile([128, kd_tiles, QL], FB)
            wvo_sb = ins.tile([128, kd_tiles, D], FB)
            wo_sb = ins.tile([128, kd_tiles, D], FB)
            ident = ins.tile([128, 128], FB)

            # unchained: per-ring FIFO keeps need-order. Chunks are ~1MB so
            # each phase's gate is a small transfer whose 16-engine
            # completion sem fires close to its byte arrival (the V8 trace
            # showed a 3MB chunk's sem lagging its bytes by ~5us). Rings
            # crawl ~100-200GB/s for their first ~1MB, so Q-critical data
            # is split across both crawl windows.
            nc.sync.dma_start(out=wq0_sb, in_=wq0[:])
            nc.sync.dma_start(out=xq123_sb, in_=xq123[:])
            nc.sync.dma_start(out=cdf_sb, in_=cdf[:])
            nc.sync.dma_start(out=wk1_sb, in_=wk1[:])
            nc.sync.dma_start(out=wv1_sb, in_=wv1[:])
            nc.sync.dma_start(out=wvo_sb, in_=wvo[:])
            nc.sync.dma_start(out=wo_sb, in_=wo[:])
            nc.scalar.dma_start(out=xq0_sb, in_=xq0[:])
            nc.scalar.dma_start(out=wq1_sb, in_=wq1[:])
            nc.scalar.dma_start(out=wk0_sb, in_=wk0[:])
            nc.scalar.dma_start(out=wv0_sb, in_=wv0[:])
            nc.scalar.dma_start(out=cod_sb, in_=cod[:])
            make_identity(nc, ident)

            # PE warm-up sized for data arrival ~11.5us (40 x ~107ns from
            # ~7.1us); also holds the PE p-state ramp before the real stream
            warm = ins.tile([128, 128], FB)
            nc.vector.memset(warm, 1.0)
            wps = psum_tr.tile([128, 128], F32, tag="tr")
            for _ in range(40):
                nc.tensor.matmul(wps, lhsT=warm, rhs=warm, start=True,
                                 stop=True)

            def x_slice(kd, col0):
                if col0 < 128:
                    return xq0_sb[:, kd, col0:col0 + 128]
                c = col0 - 128
                return xq123_sb[:, kd, c:c + 128]

            def wq_slice(kd, jb):
                w = wq0_sb if jb == 0 else wq1_sb
                return w[:, kd, :]

            def cdiff_slice(kd, col0):
                return cdf_sb[:, kd, col0:col0 + 128]

            def wk_slice(kd, jb):
                return (wk0_sb if jb == 0 else wk1_sb)[:, kd, :]

            def codd_slice(kd, col0):
                return cod_sb[:, kd, col0:col0 + 128]

            def wv_slice(kd, jb):
                return (wv0_sb if jb == 0 else wv1_sb)[:, kd, :]

            q_sb = acts.tile([128, QT, D], FB)
            kd_sb = acts.tile([128, QT, D], FB)
            v_sb = acts.tile([128, QT, D], FB)

            def mm_one(dst_tile, dst_idx, jb, lhs_fn, rhs_fn, nkd=kd_tiles):
                ps = psum.tile([128, NB], F32, tag="mm")
                for kd in range(nkd):
                    nc.tensor.matmul(
                        ps,
                        lhsT=lhs_fn(kd),
                        rhs=rhs_fn(kd, jb),
                        start=(kd == 0),
                        stop=(kd == nkd - 1),
                    )
                nc.scalar.copy(dst_tile[:, dst_idx, jb * NB:(jb + 1) * NB], ps)

            def mm_group(dst_tile, dst_idx, lhs_fn, rhs_fn):
                for jb in range(JT):
                    mm_one(dst_tile, dst_idx, jb, lhs_fn, rhs_fn)

            av_sb = acts.tile([128, QT, D], FB)

            def attention(qt):
                qv = q_sb[:, qt, :]
                kdv = kd_sb[:, qt, :]
                pe = att.tile([128, H, HD], FB, tag="prod")
                nc.vector.tensor_mul(pe.rearrange("p h e -> p (h e)"), qv, kdv)
                ds = att.tile([128, H], F32, tag="s")
                nc.vector.reduce_sum(out=ds, in_=pe, axis=mybir.AxisListType.X)
                p1 = att.tile([128, H], F32, tag="s")
                nc.scalar.activation(p1, ds,
                                     mybir.ActivationFunctionType.Sigmoid,
                                     scale=SCALE / WSCALE)
                vd = v_sb[:, qt, :].rearrange("p (h e) -> p h e", h=H)
                nc.vector.tensor_mul(
                    av_sb[:, qt, :].rearrange("p (h e) -> p h e", h=H),
                    vd, p1.to_broadcast((128, H, HD)))

            # phase order pinned (scheduler's DMA model would otherwise
            # reorder phases; see V5 post-mortem)
            with tc.tile_wait_until(1):
                for qt in range(QT):
                    mm_one(q_sb, qt, 0,
                           lambda kd, qt=qt: x_slice(kd, qt * 128), wq_slice)
            with tc.tile_wait_until(2):
                for qt in range(QT):
                    mm_one(q_sb, qt, 1,
                           lambda kd, qt=qt: x_slice(kd, qt * 128), wq_slice)
            with tc.tile_wait_until(3):
                for qt in range(QT):
                    mm_group(kd_sb, qt,
                             lambda kd, qt=qt: cdiff_slice(kd, qt * 128),
                             wk_slice)
            with tc.tile_wait_until(4):
                for qt in range(QT):
                    mm_group(v_sb, qt,
                             lambda kd, qt=qt: cdiff_slice(kd, qt * 128),
                             wv_slice)
                    if qt >= 1:
                        attention(qt - 1)
                attention(QT - 1)

            avT_sb = acts.tile([128, DT, QL], FB)

            def transposes(qt):
                for db in range(DT):
                    tp = psum_tr.tile([128, 128], FB, tag="tr")
                    nc.tensor.transpose(tp,
                                        av_sb[:, qt, db * 128:(db + 1) * 128],
                                        ident)
                    nc.scalar.copy(avT_sb[:, db, qt * 128:(qt + 1) * 128], tp)

            def o_group(qt):
                pss = [psum.tile([128, NB], F32, tag="mm", name=f"psg{jb}")
                       for jb in range(JT)]
                for jb in range(JT):
                    for kd in range(kd_tiles):
                        nc.tensor.matmul(
                            pss[jb],
                            lhsT=codd_slice(kd, qt * 128),
                            rhs=wvo_sb[:, kd, jb * NB:(jb + 1) * NB],
                            start=(kd == 0),
                            stop=False,
                        )
                    for kd in range(DT):
                        nc.tensor.matmul(
                            pss[jb],
                            lhsT=avT_sb[:, kd, qt * 128:(qt + 1) * 128],
                            rhs=wo_sb[:, kd, jb * NB:(jb + 1) * NB],
                            start=False,
                            stop=(kd == DT - 1),
                        )
                for jb in range(JT):
                    o_t = outs.tile([128, NB], F32, tag="o")
                    if jb % 2 == 0:
                        nc.scalar.copy(o_t, pss[jb])
                        nc.sync.dma_start(
                            out=out[:, qt, jb * NB:(jb + 1) * NB], in_=o_t)
                    else:
                        nc.vector.tensor_copy(o_t, pss[jb])
                        nc.scalar.dma_start(
                            out=out[:, qt, jb * NB:(jb + 1) * NB], in_=o_t)

            with tc.tile_wait_until(5):
                transposes(0)
                transposes(1)
                o_group(0)
                transposes(2)
                o_group(1)
                transposes(3)
                o_group(2)
                o_group(3)

    nc.finalize()
    return nc


_GRAPH_CACHE = {}


def _get_graph(kd_tiles: int, with_bo: bool, fp8: bool = False):
    key = (kd_tiles, with_bo, fp8)
    if key not in _GRAPH_CACHE:
        _GRAPH_CACHE[key] = _build(kd_tiles, with_bo, fp8)
    return _GRAPH_CACHE[key]


def _pmajor(a, kd_tiles):
    """[kd_tiles*128, n] -> [128, kd_tiles, n] partition-major, contiguous."""
    n = a.shape[1]
    return np.ascontiguousarray(
        a.reshape(kd_tiles, 128, n).transpose(1, 0, 2))


def _make_in_maps(x, c, Wq, bq, Wk, bk, Wv, bv, Wo, bo):
    x = np.asarray(x, np.float32)
    c = np.asarray(c, np.float32)
    has_bias = any(np.any(np.asarray(b)) for b in (bq, bk, bv))
    with_bo = bool(np.any(np.asarray(bo)))
    fp8 = not has_bias and not with_bo
    kd_tiles = DT + (1 if has_bias else 0)
    KD = kd_tiles * 128

    if fp8:
        wqT8 = _pmajor(np.ascontiguousarray(
            np.asarray(Wq, np.float32).T * WSCALE).astype(E4M3), DT)
        wq0_h = np.ascontiguousarray(wqT8[:, :, 0:NB])
        wq1_h = np.ascontiguousarray(wqT8[:, :, NB:])
        wk_h = _pmajor(np.ascontiguousarray(
            np.asarray(Wk, np.float32).T).astype(BF), DT)
        wv_h = _pmajor(np.ascontiguousarray(
            np.asarray(Wv, np.float32).T).astype(BF), DT)
        Wo32 = np.asarray(Wo, np.float32)
        wvo_h = _pmajor(np.ascontiguousarray(
            (Wo32 @ np.asarray(Wv, np.float32)).T).astype(BF), DT)
        wo_h = _pmajor(np.ascontiguousarray(Wo32.T).astype(BF), DT)
        in_maps = []
        for core in range(N_CORES):
            b = core // (N_CORES // B)
            q0 = (core % (N_CORES // B)) * QL
            k0 = 2 * q0
            xs = x[b, q0:q0 + QL]
            cs = c[b, k0:k0 + KL]
            c_odd = cs[1::2]
            c_diff = cs[0::2] - cs[1::2]
            xT_h = _pmajor(np.ascontiguousarray(xs.T).astype(BF), DT)
            codT_h = _pmajor(np.ascontiguousarray(c_odd.T).astype(BF), DT)
            cdifT_h = _pmajor(np.ascontiguousarray(c_diff.T).astype(BF), DT)
            in_maps.append({
                "wq0": wq0_h,
                "wq1": wq1_h,
                "xq0": np.ascontiguousarray(xT_h[:, :, 0:128]),
                "xq123": np.ascontiguousarray(xT_h[:, :, 128:]),
                "cdf": cdifT_h,
                "wk0": np.ascontiguousarray(wk_h[:, :, 0:NB]),
                "wk1": np.ascontiguousarray(wk_h[:, :, NB:]),
                "wv0": np.ascontiguousarray(wv_h[:, :, 0:NB]),
                "wv1": np.ascontiguousarray(wv_h[:, :, NB:]),
                "cod": codT_h,
                "wvo": wvo_h,
                "wo": wo_h,
            })
        return in_maps, kd_tiles, with_bo, True

    def aug_w(W, b):
        wT = np.asarray(W, np.float32).T          # [D, D] feature-major
        if has_bias:
            pad = np.zeros((KD - D, D), np.float32)
            pad[0, :] = np.asarray(b, np.float32)
            wT = np.concatenate([wT, pad], axis=0)
        return _pmajor(wT.astype(BF), kd_tiles)

    wq_h = aug_w(Wq, bq)
    wk_h = aug_w(Wk, bk)
    wv_h = aug_w(Wv, bv)
    # Wvo = Wo @ Wv so out = c_odd @ Wvo^T + (p1*Vd) @ Wo^T; its bias row is
    # Wo @ bv (v_odd's bias pushed through the output projection)
    Wo32 = np.asarray(Wo, np.float32)
    wvo_h = aug_w(Wo32 @ np.asarray(Wv, np.float32),
                  Wo32 @ np.asarray(bv, np.float32))
    woT = np.ascontiguousarray(Wo32.T)
    if has_bias:
        # pad wo's contraction dim to kd_tiles with zero rows so it can share
        # the woo tensor with wvo (the att-half loop only reads 8 tiles)
        woT = np.concatenate([woT, np.zeros((KD - D, D), np.float32)], axis=0)
    wo_h = _pmajor(woT.astype(BF), kd_tiles)

    def aug_act(aT, pad_val=1.0):
        # pad_val=1.0 activates the bias row of the augmented weights;
        # 0.0 for difference inputs where the bias cancels
        if has_bias:
            pad = np.zeros((KD - D, aT.shape[1]), np.float32)
            pad[0, :] = pad_val
            aT = np.concatenate([aT, pad], axis=0)
        return _pmajor(aT.astype(BF), kd_tiles)

    in_maps = []
    for core in range(N_CORES):
        b = core // (N_CORES // B)
        q0 = (core % (N_CORES // B)) * QL
        k0 = 2 * q0
        xs = x[b, q0:q0 + QL]                      # [QL, D]
        cs = c[b, k0:k0 + KL]                      # [KL, D]
        c_odd = cs[1::2]                           # [QL, D]
        c_diff = cs[0::2] - cs[1::2]               # [QL, D], fp32 exact
        xT_h = aug_act(np.ascontiguousarray(xs.T))        # [128, kd, QL]
        codT_h = aug_act(np.ascontiguousarray(c_odd.T))   # bias row active
        cdifT_h = aug_act(np.ascontiguousarray(c_diff.T), pad_val=0.0)
        X0Q = 3 * 128
        m = {
            # merged, in DMA need-order (see _build)
            "xw0": np.ascontiguousarray(
                np.concatenate([xT_h[:, :, 0:X0Q], wq_h[:, :, 0:NB]], axis=2)),
            "xw1": np.ascontiguousarray(
                np.concatenate([xT_h[:, :, X0Q:], wq_h[:, :, NB:]], axis=2)),
            "ck": np.ascontiguousarray(np.concatenate([cdifT_h, wk_h], axis=2)),
            "cv": np.ascontiguousarray(np.concatenate([codT_h, wv_h], axis=2)),
            "woo": np.ascontiguousarray(np.concatenate([wo_h, wvo_h], axis=2)),
        }
        if with_bo:
            m["bo"] = np.asarray(bo, np.float32).reshape(1, D)
        in_maps.append(m)
    return in_maps, kd_tiles, with_bo, False


def _gather(results):
    out = np.empty((B, SQ, D), np.float32)
    for core in range(N_CORES):
        b = core // (N_CORES // B)
        q0 = (core % (N_CORES // B)) * QL
        # device layout [128, QT, D] -> rows q = qt*128 + p
        arr = results[core]["out"]
        out[b, q0:q0 + QL] = arr.transpose(1, 0, 2).reshape(QL, D)
    return out


def kernel(**inputs) -> np.ndarray:
    in_maps, kd_tiles, with_bo, fp8 = _make_in_maps(**inputs)
    nc = _get_graph(kd_tiles, with_bo, fp8)
    res = run_bass_kernel_spmd(nc, in_maps, core_ids=list(range(N_CORES)))
    return _gather(res.results)


def run_traced(**inputs):
    """Like kernel() but with neuron-profile tracing; returns (out, results)."""
    in_maps, kd_tiles, with_bo, fp8 = _make_in_maps(**inputs)
    nc = _get_graph(kd_tiles, with_bo, fp8)
    res = run_bass_kernel_spmd(nc, in_maps, core_ids=list(range(N_CORES)),
                               trace=True)
    return _gather(res.results), res



# revision 5
# speedup vs baseline: 1.1222x; 1.1222x over previous
"""Sparse (block-local) attention for B=2, Sq=2048, Sk=4096, D=1024, H=16.

Each query i attends to exactly keys {2i, 2i+1} (Sk/Sq == 2, no remainder),
so softmax is over 2 scores -> p1 = sigmoid((s1-s2)*scale), p2 = 1-p1.

Distribution: sequence-parallel over (batch, query-block). 8 cores, each takes
512 contiguous queries of one batch plus the matching 1024 contiguous keys.
No collectives needed; outputs are concatenated on the host.

Algebraic cuts: with exactly 2 keys per query, softmax only needs the score
DIFFERENCE, and k_even - k_odd = (c_even - c_odd) @ Wk^T is linear, so the K
projection runs on c_diff = c_even - c_odd (512 rows, not 1024). Likewise
att = v_odd + p1 * (v_even - v_odd) reuses c_diff for V, and the v_odd term
folds through the output projection with a host-precomputed weight product
Wvo = Wo @ Wv:
  out = c_odd @ Wvo^T + (p1 * Vd) @ Wo^T,  Vd = c_diff @ Wv^T

Per-core device kernel (fp32 PSUM accumulation everywhere):
  Q  = x_s @ Wq^T     fp8 e4m3 DoubleRow (2 contraction rows/cycle, 2x rate)
  Kd = c_diff @ Wk^T  fp8 e4m3 DoubleRow
  s-diff row-wise dots on DVE per 64-dim head; p1 on ACT (sigmoid)
  Vd = c_diff @ Wv^T  bf16 (feeds the output directly -> fp8 too lossy)
  av = p1 * Vd on DVE; av^T via PE transposes
  O  = c_odd @ Wvo^T + av^T-matmul @ Wo^T   bf16

fp8 error budget (verified against an exact numpy replica): Wq/Wk ship as
e4m3 pre-scaled by 32 (folded into the sigmoid scale), x and c_diff as plain
e4m3. Only the sigmoid INPUT sees the quantization noise, so the output rel
err is ~1.6e-2 vs the 2e-2 gate; bf16 everywhere the error hits the output
linearly.

DMA: fine-grained whole tensors in phase need-order split across both hwdge
rings, no completion chains (per-ring FIFO keeps order); phase order is
pinned via tile_wait_until. Output copies split ACT/DVE into 256-col halves
DMA'd on both rings to shorten the tail.
"""

import sys

for _p in ("/opt/trn_rl_repo",):
    if _p not in sys.path:
        sys.path.append(_p)

import numpy as np
import ml_dtypes

import concourse.bass as bass
import concourse.mybir as mybir
import concourse.tile as tile
from concourse import bacc
from concourse.bass_utils import run_bass_kernel_spmd
from concourse.masks import make_identity
from concourse.tile_rust import add_dep_helper

B, SQ, SK, D, H, HD = 2, 2048, 4096, 1024, 16, 64
N_CORES = 8
QL = B * SQ // N_CORES       # 512 queries per core
KL = 2 * QL                  # 1024 keys per core
QT = QL // 128               # 4 query tiles
NB = 512                     # psum bank width (fp32)
JT = D // NB                 # 2 output-column blocks per projection
DT = D // 128                # 8 feature tiles
SCALE = 1.0 / float(np.sqrt(HD))

FB = mybir.dt.bfloat16
F32 = mybir.dt.float32
F8 = mybir.dt.float8e4
BF = ml_dtypes.bfloat16
E4M3 = ml_dtypes.float8_e4m3fn
WSCALE = 32.0


def _build(kd_tiles: int, with_bo: bool, fp8: bool = False):
    """Build + finalize the per-core Bacc graph (SPMD: same graph on 8 cores).

    fp8=True is the fast path for the bias-free case; the general
    (with-bias) path keeps everything bf16 with bias rows augmented into
    the contraction dim.
    """
    if fp8:
        return _build_fp8()
    nc = bacc.Bacc("TRN2", target_bir_lowering=False)

    # All activation/weight inputs are host-arranged partition-major:
    # tensor[p, t, n] = logical[t*128 + p, n], so DMA descriptors are
    # per-partition contiguous. Inputs are merged by NEED ORDER and the
    # DMA chain is gated so each phase gets full HBM bandwidth.
    X0Q = 3 * 128               # x columns (queries) in xw0
    xw0 = nc.dram_tensor("xw0", [128, kd_tiles, X0Q + NB], FB,
                         kind="ExternalInput")
    xw1 = nc.dram_tensor("xw1", [128, kd_tiles, (QL - X0Q) + (D - NB)], FB,
                         kind="ExternalInput")
    ck = nc.dram_tensor("ck", [128, kd_tiles, QL + D], FB, kind="ExternalInput")
    cv = nc.dram_tensor("cv", [128, kd_tiles, QL + D], FB, kind="ExternalInput")
    woo = nc.dram_tensor("woo", [128, kd_tiles, 2 * D], FB,
                         kind="ExternalInput")
    bo = None
    if with_bo:
        bo = nc.dram_tensor("bo", [1, D], F32, kind="ExternalInput")
    out = nc.dram_tensor("out", [128, QT, D], F32, kind="ExternalOutput")

    with tile.TileContext(nc) as tc:
        with (
            tc.tile_pool(name="ins", bufs=1) as ins,
            tc.tile_pool(name="acts", bufs=1) as acts,
            tc.tile_pool(name="att", bufs=4) as att,
            tc.tile_pool(name="outs", bufs=4) as outs,
            tc.tile_pool(name="psum", bufs=6, space="PSUM") as psum,
            tc.tile_pool(name="psum_tr", bufs=2, space="PSUM") as psum_tr,
        ):
            # ---- inputs to SBUF (need-order chained DMAs) ------------------
            xw0_sb = ins.tile([128, kd_tiles, X0Q + NB], FB)
            xw1_sb = ins.tile([128, kd_tiles, (QL - X0Q) + (D - NB)], FB)
            ck_sb = ins.tile([128, kd_tiles, QL + D], FB)
            cv_sb = ins.tile([128, kd_tiles, QL + D], FB)
            woo_sb = ins.tile([128, kd_tiles, 2 * D], FB)
            ident = ins.tile([128, 128], FB)

            h0 = (X0Q + NB) // 2
            d0a = nc.sync.dma_start(out=xw0_sb[:, :, 0:h0], in_=xw0[:, :, 0:h0])
            d0b = nc.scalar.dma_start(out=xw0_sb[:, :, h0:], in_=xw0[:, :, h0:])
            d1 = nc.sync.dma_start(out=xw1_sb, in_=xw1[:])
            d2 = nc.sync.dma_start(out=ck_sb, in_=ck[:])
            d3 = nc.sync.dma_start(out=cv_sb, in_=cv[:])
            d4 = nc.sync.dma_start(out=woo_sb, in_=woo[:])
            for d0x in (d0a, d0b):
                add_dep_helper(d1.ins, d0x.ins, sync=True)
                add_dep_helper(d2.ins, d0x.ins, sync=True)
            add_dep_helper(d3.ins, d1.ins, sync=True)
            add_dep_helper(d3.ins, d2.ins, sync=True)
            add_dep_helper(d4.ins, d3.ins, sync=True)
            bo_sb = None
            if with_bo:
                bo_sb = ins.tile([128, D], F32)
                d5 = nc.sync.dma_start(out=bo_sb,
                                       in_=bo[:].to_broadcast((128, D)))
                add_dep_helper(d5.ins, d3.ins, sync=True)
            make_identity(nc, ident)

            # PE warm-up: dummy matmuls during the DMA head keep HAM busy so
            # the real stream starts at full clock, at zero wall-clock cost.
            warm = ins.tile([128, 128], FB)
            nc.vector.memset(warm, 1.0)
            wps = psum_tr.tile([128, 128], F32, tag="tr")
            for _ in range(110):
                nc.tensor.matmul(wps, lhsT=warm, rhs=warm, start=True, stop=True)

            def x_slice(kd, col0):
                if col0 < X0Q:
                    return xw0_sb[:, kd, col0:col0 + 128]
                c = col0 - X0Q
                return xw1_sb[:, kd, c:c + 128]

            def wq_slice(kd, jb):
                if jb == 0:
                    return xw0_sb[:, kd, X0Q:X0Q + NB]
                c = (QL - X0Q) + (jb - 1) * NB
                return xw1_sb[:, kd, c:c + NB]

            def cdiff_slice(kd, col0):
                return ck_sb[:, kd, col0:col0 + 128]

            def wk_slice(kd, jb):
                return ck_sb[:, kd, QL + jb * NB:QL + (jb + 1) * NB]

            def codd_slice(kd, col0):
                return cv_sb[:, kd, col0:col0 + 128]

            def wv_slice(kd, jb):
                return cv_sb[:, kd, QL + jb * NB:QL + (jb + 1) * NB]

            # ---- projections (psum copies all on ACT) ----------------------
            q_sb = acts.tile([128, QT, D], FB)           # Q row-major
            kd_sb = acts.tile([128, QT, D], FB)          # Kd = c_diff @ Wk^T
            v_sb = acts.tile([128, QT, D], FB)           # Vd = c_diff @ Wv^T

            def mm_one(dst_tile, dst_idx, jb, lhs_fn, rhs_fn, nkd=kd_tiles):
                ps = psum.tile([128, NB], F32, tag="mm")
                for kd in range(nkd):
                    nc.tensor.matmul(
                        ps,
                        lhsT=lhs_fn(kd),
                        rhs=rhs_fn(kd, jb),
                        start=(kd == 0),
                        stop=(kd == nkd - 1),
                    )
                nc.scalar.copy(dst_tile[:, dst_idx, jb * NB:(jb + 1) * NB], ps)

            def mm_group(dst_tile, dst_idx, lhs_fn, rhs_fn):
                for jb in range(JT):
                    mm_one(dst_tile, dst_idx, jb, lhs_fn, rhs_fn)

            av_sb = acts.tile([128, QT, D], FB)

            def attention(qt):
                qv = q_sb[:, qt, :]
                kdv = kd_sb[:, qt, :]
                pe = att.tile([128, H, HD], FB, tag="prod")
                nc.vector.tensor_mul(pe.rearrange("p h e -> p (h e)"), qv, kdv)
                ds = att.tile([128, H], F32, tag="s")
                nc.vector.reduce_sum(out=ds, in_=pe, axis=mybir.AxisListType.X)
                p1 = att.tile([128, H], F32, tag="s")
                nc.scalar.activation(p1, ds, mybir.ActivationFunctionType.Sigmoid,
                                     scale=SCALE)
                vd = v_sb[:, qt, :].rearrange("p (h e) -> p h e", h=H)
                nc.vector.tensor_mul(
                    av_sb[:, qt, :].rearrange("p (h e) -> p h e", h=H),
                    vd, p1.to_broadcast((128, H, HD)))

            for jb in range(JT):
                for qt in range(QT):
                    mm_one(q_sb, qt, jb,
                           lambda kd, qt=qt: x_slice(kd, qt * 128), wq_slice)
            for qt in range(QT):
                mm_group(kd_sb, qt,
                         lambda kd, qt=qt: cdiff_slice(kd, qt * 128), wk_slice)
            for qt in range(QT):
                mm_group(v_sb, qt,
                         lambda kd, qt=qt: cdiff_slice(kd, qt * 128), wv_slice)
                if qt >= 1:
                    attention(qt - 1)
            attention(QT - 1)

            # ---- transpose att -> attT (copies on ACT), O groups interleaved
            avT_sb = acts.tile([128, DT, QL], FB)        # att^T feature-major

            def transposes(qt):
                for db in range(DT):
                    tp = psum_tr.tile([128, 128], FB, tag="tr")
                    nc.tensor.transpose(tp, av_sb[:, qt, db * 128:(db + 1) * 128],
                                        ident)
                    nc.scalar.copy(avT_sb[:, db, qt * 128:(qt + 1) * 128], tp)

            def o_group(qt):
                pss = [psum.tile([128, NB], F32, tag="mm", name=f"psg{jb}") for jb in range(JT)]
                for jb in range(JT):
                    for kd in range(kd_tiles):
                        nc.tensor.matmul(
                            pss[jb],
                            lhsT=codd_slice(kd, qt * 128),
                            rhs=woo_sb[:, kd, D + jb * NB:D + (jb + 1) * NB],
                            start=(kd == 0),
                            stop=False,
                        )
                    for kd in range(DT):
                        nc.tensor.matmul(
                            pss[jb],
                            lhsT=avT_sb[:, kd, qt * 128:(qt + 1) * 128],
                            rhs=woo_sb[:, kd, jb * NB:(jb + 1) * NB],
                            start=False,
                            stop=(kd == DT - 1),
                        )
                for jb in range(JT):
                    o_t = outs.tile([128, NB], F32, tag="o")
                    if with_bo:
                        nc.vector.tensor_add(o_t, pss[jb],
                                             bo_sb[:, jb * NB:(jb + 1) * NB])
                    elif jb % 2 == 0:
                        nc.scalar.copy(o_t, pss[jb])
                    else:
                        nc.vector.tensor_copy(o_t, pss[jb])
                    nc.sync.dma_start(out=out[:, qt, jb * NB:(jb + 1) * NB],
                                      in_=o_t)

            transposes(0)
            transposes(1)
            o_group(0)
            transposes(2)
            o_group(1)
            transposes(3)
            o_group(2)
            o_group(3)

    nc.finalize()
    return nc


def _build_fp8():
    """Bias-free fast path: fp8 DoubleRow Q/Kd, bf16 Vd/O."""
    nc = bacc.Bacc("TRN2", target_bir_lowering=False)
    kd_tiles = DT
    DR = mybir.MatmulPerfMode.DoubleRow

    xq0 = nc.dram_tensor("xq0", [128, kd_tiles, 128], F8, kind="ExternalInput")
    xq123 = nc.dram_tensor("xq123", [128, kd_tiles, QL - 128], F8,
                           kind="ExternalInput")
    wq0a = nc.dram_tensor("wq0a", [128, kd_tiles, NB // 2], F8,
                          kind="ExternalInput")
    wq0b = nc.dram_tensor("wq0b", [128, kd_tiles, NB // 2], F8,
                          kind="ExternalInput")
    wq1 = nc.dram_tensor("wq1", [128, kd_tiles, NB], F8, kind="ExternalInput")
    cdf8 = nc.dram_tensor("cdf8", [128, kd_tiles, QL], F8,
                          kind="ExternalInput")
    wk0 = nc.dram_tensor("wk0", [128, kd_tiles, NB], F8, kind="ExternalInput")
    wk1 = nc.dram_tensor("wk1", [128, kd_tiles, NB], F8, kind="ExternalInput")
    cdf = nc.dram_tensor("cdf", [128, kd_tiles, QL], FB, kind="ExternalInput")
    wv0 = nc.dram_tensor("wv0", [128, kd_tiles, NB], FB, kind="ExternalInput")
    wv1 = nc.dram_tensor("wv1", [128, kd_tiles, NB], FB, kind="ExternalInput")
    cod = nc.dram_tensor("cod", [128, kd_tiles, QL], FB, kind="ExternalInput")
    wvo = nc.dram_tensor("wvo", [128, kd_tiles, D], FB, kind="ExternalInput")
    wo = nc.dram_tensor("wo", [128, kd_tiles, D], FB, kind="ExternalInput")
    out = nc.dram_tensor("out", [128, QT, D], F32, kind="ExternalOutput")

    with tile.TileContext(nc) as tc:
        with (
            tc.tile_pool(name="ins", bufs=1) as ins,
            tc.tile_pool(name="acts", bufs=1) as acts,
            tc.tile_pool(name="att", bufs=4) as att,
            tc.tile_pool(name="outs", bufs=8) as outs,
            tc.tile_pool(name="psum", bufs=5, space="PSUM") as psum,
            tc.tile_pool(name="psum_tr", bufs=2, space="PSUM") as psum_tr,
            tc.tile_pool(name="psum_w", bufs=1, space="PSUM") as psum_w,
        ):
            xq0_sb = ins.tile([128, kd_tiles, 128], F8)
            xq123_sb = ins.tile([128, kd_tiles, QL - 128], F8)
            wq0a_sb = ins.tile([128, kd_tiles, NB // 2], F8)
            wq0b_sb = ins.tile([128, kd_tiles, NB // 2], F8)
            wq1_sb = ins.tile([128, kd_tiles, NB], F8)
            cdf8_sb = ins.tile([128, kd_tiles, QL], F8)
            wk0_sb = ins.tile([128, kd_tiles, NB], F8)
            wk1_sb = ins.tile([128, kd_tiles, NB], F8)
            cdf_sb = ins.tile([128, kd_tiles, QL], FB)
            wv0_sb = ins.tile([128, kd_tiles, NB], FB)
            wv1_sb = ins.tile([128, kd_tiles, NB], FB)
            cod_sb = ins.tile([128, kd_tiles, QL], FB)
            wvo_sb = ins.tile([128, kd_tiles, D], FB)
            wo_sb = ins.tile([128, kd_tiles, D], FB)
            ident = ins.tile([128, 128], FB)

            # unchained: per-ring FIFO keeps need-order. Q-critical data is
            # split across both rings' crawl windows; each later phase's
            # tensors are balanced so both rings deliver it about when the
            # PE stream reaches it.
            nc.sync.dma_start(out=wq0a_sb, in_=wq0a[:])
            nc.sync.dma_start(out=xq123_sb, in_=xq123[:])
            nc.sync.dma_start(out=wk0_sb, in_=wk0[:])
            nc.sync.dma_start(out=cdf_sb, in_=cdf[:])
            nc.sync.dma_start(out=wv1_sb, in_=wv1[:])
            nc.sync.dma_start(out=cod_sb, in_=cod[:])
            nc.sync.dma_start(out=wo_sb, in_=wo[:])
            nc.scalar.dma_start(out=xq0_sb, in_=xq0[:])
            nc.scalar.dma_start(out=wq0b_sb, in_=wq0b[:])
            nc.scalar.dma_start(out=wq1_sb, in_=wq1[:])
            nc.scalar.dma_start(out=cdf8_sb, in_=cdf8[:])
            nc.scalar.dma_start(out=wk1_sb, in_=wk1[:])
            nc.scalar.dma_start(out=wv0_sb, in_=wv0[:])
            nc.scalar.dma_start(out=wvo_sb, in_=wvo[:])
            make_identity(nc, ident)

            # PE warm-up holds the p-state ramp until the first Q data lands
            warm = ins.tile([128, 128], FB)
            nc.vector.memset(warm, 1.0)
            wps = psum_w.tile([128, 128], F32, tag="warm")
            for _ in range(44):
                nc.tensor.matmul(wps, lhsT=warm, rhs=warm, start=True,
                                 stop=True)

            def x2(t, qt):
                # fp8 DoubleRow lhsT: contraction pair (2t, 2t+1), 128 q cols
                if qt == 0:
                    return xq0_sb[:, 2 * t:2 * t + 2, :]
                c = (qt - 1) * 128
                return xq123_sb[:, 2 * t:2 * t + 2, c:c + 128]

            def cdiff8_2(t, qt):
                return cdf8_sb[:, 2 * t:2 * t + 2, qt * 128:(qt + 1) * 128]

            def cdiff_slice(kd, col0):
                return cdf_sb[:, kd, col0:col0 + 128]

            def codd_slice(kd, col0):
                return cod_sb[:, kd, col0:col0 + 128]

            def wv_slice(kd, jb):
                return (wv0_sb if jb == 0 else wv1_sb)[:, kd, :]

            q_sb = acts.tile([128, QT, D], FB)
            kd_sb = acts.tile([128, QT, D], FB)
            v_sb = acts.tile([128, QT, D], FB)

            def mm_dr(dst_tile, dst_idx, jb, lhs_fn, rhs_sb):
                # 4 DoubleRow matmuls, 256-contraction each
                ps = psum.tile([128, NB], F32, tag="mm")
                for t in range(kd_tiles // 2):
                    nc.tensor.matmul(
                        ps,
                        lhsT=lhs_fn(t),
                        rhs=rhs_sb[:, 2 * t:2 * t + 2, :],
                        start=(t == 0),
                        stop=(t == kd_tiles // 2 - 1),
                        perf_mode=DR,
                    )
                nc.scalar.copy(dst_tile[:, dst_idx, jb * NB:(jb + 1) * NB], ps)

            def mm_one(dst_tile, dst_idx, jb, lhs_fn, rhs_fn, nkd=kd_tiles):
                ps = psum.tile([128, NB], F32, tag="mm")
                for kd in range(nkd):
                    nc.tensor.matmul(
                        ps,
                        lhsT=lhs_fn(kd),
                        rhs=rhs_fn(kd, jb),
                        start=(kd == 0),
                        stop=(kd == nkd - 1),
                    )
                nc.scalar.copy(dst_tile[:, dst_idx, jb * NB:(jb + 1) * NB], ps)

            av_sb = acts.tile([128, QT, D], FB)

            def attention(qt):
                qv = q_sb[:, qt, :]
                kdv = kd_sb[:, qt, :]
                pe = att.tile([128, H, HD], FB, tag="prod")
                nc.vector.tensor_mul(pe.rearrange("p h e -> p (h e)"), qv, kdv)
                ds = att.tile([128, H], F32, tag="s")
                nc.vector.reduce_sum(out=ds, in_=pe, axis=mybir.AxisListType.X)
                p1 = att.tile([128, H], F32, tag="s")
                nc.scalar.activation(p1, ds,
                                     mybir.ActivationFunctionType.Sigmoid,
                                     scale=SCALE / (WSCALE * WSCALE))
                vd = v_sb[:, qt, :].rearrange("p (h e) -> p h e", h=H)
                nc.vector.tensor_mul(
                    av_sb[:, qt, :].rearrange("p (h e) -> p h e", h=H),
                    vd, p1.to_broadcast((128, H, HD)))

            # Q jb0 in 256-col halves so the first groups gate on the
            # smallest possible front transfers (wq0a on sync, wq0b+xq0
            # on scalar); both halves accumulate in one psum bank
            with tc.tile_wait_until(1):
                for qt in range(QT):
                    ps = psum.tile([128, NB], F32, tag="mm")
                    nh = NB // 2
                    for h in range(2):
                        w_sb = wq0a_sb if h == 0 else wq0b_sb
                        for t in range(kd_tiles // 2):
                            nc.tensor.matmul(
                                ps[:, h * nh:(h + 1) * nh],
                                lhsT=x2(t, qt),
                                rhs=w_sb[:, 2 * t:2 * t + 2, :],
                                start=(t == 0),
                                stop=(t == kd_tiles // 2 - 1),
                                perf_mode=DR,
                            )
                    nc.scalar.copy(q_sb[:, qt, 0:NB], ps)
            with tc.tile_wait_until(2):
                for qt in range(QT):
                    mm_dr(q_sb, qt, 1, lambda t, qt=qt: x2(t, qt), wq1_sb)
            with tc.tile_wait_until(3):
                for qt in range(QT):
                    mm_dr(kd_sb, qt, 0,
                          lambda t, qt=qt: cdiff8_2(t, qt), wk0_sb)
                    mm_dr(kd_sb, qt, 1,
                          lambda t, qt=qt: cdiff8_2(t, qt), wk1_sb)
            with tc.tile_wait_until(4):
                for qt in range(QT):
                    for jb in range(JT):
                        mm_one(v_sb, qt, jb,
                               lambda kd, qt=qt: cdiff_slice(kd, qt * 128),
                               wv_slice)
                    if qt >= 1:
                        attention(qt - 1)
                attention(QT - 1)

            avT_sb = acts.tile([128, DT, QL], FB)

            def transposes(qt):
                for db in range(DT):
                    tp = psum_tr.tile([128, 128], FB, tag="tr")
                    nc.tensor.transpose(tp,
                                        av_sb[:, qt, db * 128:(db + 1) * 128],
                                        ident)
                    nc.scalar.copy(avT_sb[:, db, qt * 128:(qt + 1) * 128], tp)

            def o_group(qt):
                pss = [psum.tile([128, NB], F32, tag="mm", name=f"psg{jb}")
                       for jb in range(JT)]
                for jb in range(JT):
                    for kd in range(kd_tiles):
                        nc.tensor.matmul(
                            pss[jb],
                            lhsT=codd_slice(kd, qt * 128),
                            rhs=wvo_sb[:, kd, jb * NB:(jb + 1) * NB],
                            start=(kd == 0),
                            stop=False,
                        )
                    for kd in range(DT):
                        nc.tensor.matmul(
                            pss[jb],
                            lhsT=avT_sb[:, kd, qt * 128:(qt + 1) * 128],
                            rhs=wo_sb[:, kd, jb * NB:(jb + 1) * NB],
                            start=False,
                            stop=(kd == DT - 1),
                        )
                # copies split ACT/DVE into 256-col halves, DMA'd on both
                # rings: halves the post-last-matmul tail
                for jb in range(JT):
                    o_t = outs.tile([128, NB], F32, tag="o")
                    nh = NB // 2
                    nc.scalar.copy(o_t[:, 0:nh], pss[jb][:, 0:nh])
                    nc.vector.tensor_copy(o_t[:, nh:], pss[jb][:, nh:])
                    nc.sync.dma_start(
                        out=out[:, qt, jb * NB:jb * NB + nh], in_=o_t[:, 0:nh])
                    nc.scalar.dma_start(
                        out=out[:, qt, jb * NB + nh:(jb + 1) * NB],
                        in_=o_t[:, nh:])

            with tc.tile_wait_until(5):
                transposes(0)
                transposes(1)
                o_group(0)
                transposes(2)
                o_group(1)
                transposes(3)
                o_group(2)
                o_group(3)

    nc.finalize()
    return nc


_GRAPH_CACHE = {}


def _get_graph(kd_tiles: int, with_bo: bool, fp8: bool = False):
    key = (kd_tiles, with_bo, fp8)
    if key not in _GRAPH_CACHE:
        _GRAPH_CACHE[key] = _build(kd_tiles, with_bo, fp8)
    return _GRAPH_CACHE[key]


def _pmajor(a, kd_tiles):
    """[kd_tiles*128, n] -> [128, kd_tiles, n] partition-major, contiguous."""
    n = a.shape[1]
    return np.ascontiguousarray(
        a.reshape(kd_tiles, 128, n).transpose(1, 0, 2))


def _make_in_maps(x, c, Wq, bq, Wk, bk, Wv, bv, Wo, bo):
    x = np.asarray(x, np.float32)
    c = np.asarray(c, np.float32)
    has_bias = any(np.any(np.asarray(b)) for b in (bq, bk, bv))
    with_bo = bool(np.any(np.asarray(bo)))
    fp8 = not has_bias and not with_bo
    kd_tiles = DT + (1 if has_bias else 0)
    KD = kd_tiles * 128

    if fp8:
        wqT8 = _pmajor(np.ascontiguousarray(
            np.asarray(Wq, np.float32).T * WSCALE).astype(E4M3), DT)
        wkT8 = _pmajor(np.ascontiguousarray(
            np.asarray(Wk, np.float32).T * WSCALE).astype(E4M3), DT)
        wv_h = _pmajor(np.ascontiguousarray(
            np.asarray(Wv, np.float32).T).astype(BF), DT)
        Wo32 = np.asarray(Wo, np.float32)
        wvo_h = _pmajor(np.ascontiguousarray(
            (Wo32 @ np.asarray(Wv, np.float32)).T).astype(BF), DT)
        wo_h = _pmajor(np.ascontiguousarray(Wo32.T).astype(BF), DT)
        nh = NB // 2
        shared = {
            "wq0a": np.ascontiguousarray(wqT8[:, :, 0:nh]),
            "wq0b": np.ascontiguousarray(wqT8[:, :, nh:NB]),
            "wq1": np.ascontiguousarray(wqT8[:, :, NB:]),
            "wk0": np.ascontiguousarray(wkT8[:, :, 0:NB]),
            "wk1": np.ascontiguousarray(wkT8[:, :, NB:]),
            "wv0": np.ascontiguousarray(wv_h[:, :, 0:NB]),
            "wv1": np.ascontiguousarray(wv_h[:, :, NB:]),
            "wvo": wvo_h,
            "wo": wo_h,
        }
        in_maps = []
        for core in range(N_CORES):
            b = core // (N_CORES // B)
            q0 = (core % (N_CORES // B)) * QL
            k0 = 2 * q0
            xs = x[b, q0:q0 + QL]
            cs = c[b, k0:k0 + KL]
            c_odd = cs[1::2]
            c_diff = cs[0::2] - cs[1::2]
            xT8 = _pmajor(np.ascontiguousarray(xs.T).astype(E4M3), DT)
            cdifT = np.ascontiguousarray(c_diff.T)
            m = dict(shared)
            m.update({
                "xq0": np.ascontiguousarray(xT8[:, :, 0:128]),
                "xq123": np.ascontiguousarray(xT8[:, :, 128:]),
                "cdf8": _pmajor(cdifT.astype(E4M3), DT),
                "cdf": _pmajor(cdifT.astype(BF), DT),
                "cod": _pmajor(np.ascontiguousarray(c_odd.T).astype(BF), DT),
            })
            in_maps.append(m)
        return in_maps, kd_tiles, with_bo, True

    def aug_w(W, b):
        wT = np.asarray(W, np.float32).T          # [D, D] feature-major
        if has_bias:
            pad = np.zeros((KD - D, D), np.float32)
            pad[0, :] = np.asarray(b, np.float32)
            wT = np.concatenate([wT, pad], axis=0)
        return _pmajor(wT.astype(BF), kd_tiles)

    wq_h = aug_w(Wq, bq)
    wk_h = aug_w(Wk, bk)
    wv_h = aug_w(Wv, bv)
    Wo32 = np.asarray(Wo, np.float32)
    wvo_h = aug_w(Wo32 @ np.asarray(Wv, np.float32),
                  Wo32 @ np.asarray(bv, np.float32))
    woT = np.ascontiguousarray(Wo32.T)
    if has_bias:
        woT = np.concatenate([woT, np.zeros((KD - D, D), np.float32)], axis=0)
    wo_h = _pmajor(woT.astype(BF), kd_tiles)

    def aug_act(aT, pad_val=1.0):
        if has_bias:
            pad = np.zeros((KD - D, aT.shape[1]), np.float32)
            pad[0, :] = pad_val
            aT = np.concatenate([aT, pad], axis=0)
        return _pmajor(aT.astype(BF), kd_tiles)

    in_maps = []
    for core in range(N_CORES):
        b = core // (N_CORES // B)
        q0 = (core % (N_CORES // B)) * QL
        k0 = 2 * q0
        xs = x[b, q0:q0 + QL]                      # [QL, D]
        cs = c[b, k0:k0 + KL]                      # [KL, D]
        c_odd = cs[1::2]                           # [QL, D]
        c_diff = cs[0::2] - cs[1::2]               # [QL, D], fp32 exact
        xT_h = aug_act(np.ascontiguousarray(xs.T))        # [128, kd, QL]
        codT_h = aug_act(np.ascontiguousarray(c_odd.T))   # bias row active
        cdifT_h = aug_act(np.ascontiguousarray(c_diff.T), pad_val=0.0)
        X0Q = 3 * 128
        m = {
            "xw0": np.ascontiguousarray(
                np.concatenate([xT_h[:, :, 0:X0Q], wq_h[:, :, 0:NB]], axis=2)),
            "xw1": np.ascontiguousarray(
                np.concatenate([xT_h[:, :, X0Q:], wq_h[:, :, NB:]], axis=2)),
            "ck": np.ascontiguousarray(np.concatenate([cdifT_h, wk_h], axis=2)),
            "cv": np.ascontiguousarray(np.concatenate([codT_h, wv_h], axis=2)),
            "woo": np.ascontiguousarray(np.concatenate([wo_h, wvo_h], axis=2)),
        }
        if with_bo:
            m["bo"] = np.asarray(bo, np.float32).reshape(1, D)
        in_maps.append(m)
    return in_maps, kd_tiles, with_bo, False


def _gather(results):
    out = np.empty((B, SQ, D), np.float32)
    for core in range(N_CORES):
        b = core // (N_CORES // B)
        q0 = (core % (N_CORES // B)) * QL
        arr = results[core]["out"]
        out[b, q0:q0 + QL] = arr.transpose(1, 0, 2).reshape(QL, D)
    return out


def kernel(**inputs) -> np.ndarray:
    in_maps, kd_tiles, with_bo, fp8 = _make_in_maps(**inputs)
    nc = _get_graph(kd_tiles, with_bo, fp8)
    res = run_bass_kernel_spmd(nc, in_maps, core_ids=list(range(N_CORES)))
    return _gather(res.results)


def run_traced(**inputs):
    """Like kernel() but with neuron-profile tracing; returns (out, results)."""
    in_maps, kd_tiles, with_bo, fp8 = _make_in_maps(**inputs)
    nc = _get_graph(kd_tiles, with_bo, fp8)
    res = run_bass_kernel_spmd(nc, in_maps, core_ids=list(range(N_CORES)),
                               trace=True)
    return _gather(res.results), res
